# revision 85
# baseline (speedup 1.0000x reference)
"""DeepEmbedAttention TRN2 kernel — 8-core SPMD, v2.

Sharding: 2 cores per batch (B=4). Each core computes the full k/v chain for
its batch (T=2048) and attention outputs for 4 query chunks of 256 tokens.
Chunk assignment is causally load-balanced: even cores take chunks {0,3,4,7},
odd cores {1,2,5,6}. The single SPMD program processes chunks at canonical
slot positions; everything position-dependent (q columns, causal masks,
chunk-boundary tokens) arrives as per-core input data, so one program serves
all 8 cores. Softmax needs no max-subtraction: scores are tanh-capped to
[-64, 64], so exp() cannot overflow fp32.

v2 engine plan (from sim-profile of v1: ACT 73% busy was the bottleneck,
with ~40us of DMA issue + ~31us of mean/var accum passes riding the scalar
queue):
 - All activations stay inside the one co-resident ACT table set
   (exp_and_others: tanh/exp/square/copy) — single table load.
 - v-chain layernorm stats fused into DVE passes: blend-add carries the
   row-sum (scalar_tensor_tensor accum_out), sum-of-squares is one
   tensor_tensor_reduce pass. Nothing on ACT but the tanh.
 - token-shift: superdiagonal PE matmul for rows 1..127; row 0 (the carry
   from the previous tile) is seeded into PSUM by a 1-row gpsimd copy with
   mismatched partition bases instead of a full-width boundary matmul.
 - rsqrt for layernorm runs as batched magic-rsqrt epochs (groups of tiles)
   instead of 40 per-tile call sites.
 - Attention is interleaved with the v chain: slot s is emitted as soon as
   v tiles < 4(s+1) exist. PSUM budget: v-chain 2 banks (single-buffered
   pool holds vps then vshps) + scores 1 + sums 1 + out 4 = 8.
 - DMA issue is spread across the three issuing queues (sync / scalar /
   gpsimd); output is stored bf16 and widened on host.
"""

import sys

if "/opt/trn_rl_repo" not in sys.path:
    sys.path.insert(0, "/opt/trn_rl_repo")

import numpy as np

B, T, C = 4, 2048, 1024
QD, KV = 256, 32
SCORE_SCALE, CAP_SCALE = 1024.0, 64.0
EPS = 1e-5
N_CORES = 8
P = 128
CHUNK = 256
NSLOT = 4                       # q-chunks per core
TQ = NSLOT * CHUNK              # 1024 canonical query tokens per core
NT = T // P                     # 16 token tiles (full sequence)
NQT = TQ // P                   # 8 canonical query token tiles
CHUNKS = [[0, 3, 4, 7], [1, 2, 5, 6]]   # parity -> global chunk ids
R = [4, 8, 12, 16]              # k-tiles per slot (max over parities)
MINQS = [0, 512, 1024, 1536]    # min chunk start over parities, per slot
NEED_MASK = [(s, kt) for s in range(NSLOT) for kt in range(R[s])
             if P * (kt + 1) > MINQS[s]]
MASK_IDX = {sk: i for i, sk in enumerate(NEED_MASK)}
NMASK = len(NEED_MASK)          # 16
NEG = -1.0e30


def _build_program(nc, tc, a, apply_gb, bf16, nrep=1, phases=4):
    from contextlib import ExitStack

    import concourse.mybir as mybir
    from concourse.masks import make_identity

    f32 = mybir.dt.float32
    i32 = mybir.dt.int32
    DT = mybir.dt.bfloat16 if bf16 else f32
    Alu = mybir.AluOpType
    Act = mybir.ActivationFunctionType

    xTr = a["xT"].rearrange("(a p) t -> p a t", p=P)        # [128, 8, 2048]
    xqTr = a["xqT"].rearrange("(a p) t -> p a t", p=P)      # [128, 8, 1024]
    xqpr = a["xqprevT"].rearrange("(a p) t -> p a t", p=P)  # [128, 8, 4]
    wqqr = a["wqq"].rearrange("(a p) d -> p a d", p=P)      # [128, 8, 256]
    wkvr = a["wkv"].rearrange("(a p) d -> p a d", p=P)      # [128, 8, 64]
    kembr = a["kemb"].rearrange("(g p) d -> p g d", p=P)    # [128, 16, 256]
    vembr1 = a["vemb1"].rearrange("(g p) d -> p g d", p=P)  # [128, 16, 1024]
    vembr2 = a["vemb2"].rearrange("(g p) d -> p g d", p=P)
    maskr = a["mask"].rearrange("m p q -> p m q")           # [128, 16, 256]
    out_d = a["out"]                                        # [1024, 1024] bf16

    ctx = ExitStack()
    const = ctx.enter_context(tc.tile_pool(name="const", bufs=1))
    pers = ctx.enter_context(tc.tile_pool(name="pers", bufs=1))

    # --- constants (gpsimd queue for the DMAs) ---
    ident = const.tile([P, P], DT, tag="ident")
    make_identity(nc, ident[:])
    # ssup[p, m] = 1 iff m == p+1 : shift-down-one (sh[m] = v[m-1]), row 0 = 0
    ssup = const.tile([P, P], DT, tag="ssup")
    nc.gpsimd.memset(ssup[:], 0.0)
    nc.gpsimd.affine_select(out=ssup[:], in_=ssup[:],
                            compare_op=Alu.not_equal, fill=1.0,
                            base=1, pattern=[[-1, P]], channel_multiplier=1)
    # bnd[p, m] = 1 iff (p==127, m==0) : carry prev tile's last row into row 0
    bnd = const.tile([P, P], DT, tag="bnd")
    nc.gpsimd.memset(bnd[:], 0.0)
    nc.gpsimd.affine_select(out=bnd[:], in_=bnd[:],
                            compare_op=Alu.not_equal, fill=1.0,
                            base=-(P - 1), pattern=[[-P, P]],
                            channel_multiplier=1)
    # qsel[s][p, m] = 1 iff (p==s, m==0) : qprev row s into row 0
    qsel = []
    for s in range(NSLOT):
        qs_t = const.tile([NSLOT, P], DT, tag=f"qsel{s}", name=f"qsel{s}")
        nc.gpsimd.memset(qs_t[:], 0.0)
        nc.gpsimd.affine_select(out=qs_t[:], in_=qs_t[:],
                                compare_op=Alu.not_equal, fill=1.0,
                                base=-s, pattern=[[-NSLOT, P]],
                                channel_multiplier=1)
        qsel.append(qs_t)
    ones1 = const.tile([P, 1], DT, tag="ones1")
    nc.gpsimd.memset(ones1[:], 1.0)

    # weights that gate phase-1 compute go first, on the fast HWDGE queues
    wkv = const.tile([P, 8, 64], DT, tag="wkv")
    nc.sync.dma_start(wkv[:], wkvr[:])
    wqq = const.tile([P, 8, QD], DT, tag="wqq")
    nc.scalar.dma_start(wqq[:], wqqr[:])
    wkup = const.tile([KV, QD], DT, tag="wkup")
    nc.gpsimd.dma_start(wkup[:], a["wkup"][:])
    # v_mid lives at base partition 32 inside kvmid; PE needs lhsT/rhs bases
    # to match, so W_vupT is loaded at partitions 32..63 as well.
    wvup64 = const.tile([64, C], DT, tag="wvup")
    nc.gpsimd.dma_start(wvup64[KV:64, :], a["wvup"][:])
    wvup = wvup64[KV:64, :]
    xq_rep = const.tile([P, QD], DT, tag="xq_rep")
    nc.gpsimd.dma_start(xq_rep[:], a["xq_rep"][:])
    xk_rep = const.tile([P, QD], DT, tag="xk_rep")
    nc.gpsimd.dma_start(xk_rep[:], a["xk_rep"][:])
    maskall = const.tile([P, NMASK, CHUNK], DT, tag="maskall")
    nc.gpsimd.dma_start(maskall[:], maskr[:])
    gb = {}
    if apply_gb:
        for nm, d in [("gq", QD), ("bq", QD), ("gk", QD), ("bk", QD),
                      ("gv", C), ("bv", C)]:
            gb[nm] = const.tile([P, d], DT, tag=nm + "_rep", name=nm + "_rep")
            nc.gpsimd.dma_start(gb[nm][:], a[nm + "_rep"][:])

    loop = tc.For_i(0, nrep, 1) if nrep > 1 else None
    if loop is not None:
        loop.__enter__()

    # --- persistent strips ---
    kvmid = pers.tile([64, T], DT, tag="kvmid")       # [k_mid; v_mid]^T
    qraw = pers.tile([P, NQT, QD], DT, tag="qraw")    # canonical q tiles
    qprev = pers.tile([NSLOT, QD], DT, tag="qprev")   # chunk-boundary q rows
    kk = pers.tile([P, NT, QD], DT, tag="kk")         # pre-blend k
    kf = pers.tile([P, NT, QD], DT, tag="kf")         # post-LN k
    qf = pers.tile([P, NQT, QD], DT, tag="qf")        # post-LN q
    vv = pers.tile([P, NT, C], DT, tag="vv")          # post-LN v
    kT = pers.tile([P, 2, T], DT, tag="kT")           # k^T for attention
    qT = pers.tile([P, 2, TQ], DT, tag="qT")          # q^T for attention
    # layernorm stats strips: [sum; sumsq] per tile, finished in rsqrt epochs
    kst = pers.tile([P, NT, 2], f32, tag="kst")
    qst = pers.tile([P, NQT, 2], f32, tag="qst")
    # v stats: [sum_h0, sum_h1, ssq_h0, ssq_h1] -> [bias, rstd, _, _]
    vst = pers.tile([P, NT, 4], f32, tag="vst")

    gq, bq = (gb.get("gq"), gb.get("bq"))
    gk, bk = (gb.get("gk"), gb.get("bk"))
    gv, bv = (gb.get("gv"), gb.get("bv"))

    def batch_rsqrt(st, lo, n, pool, nm, inv_w=None, eng=None):
        # st[:, lo:lo+n, 1] <- rsqrt(var + EPS) via magic-constant seed + 2
        # Newton iterations, batched over n tiles (on GpSimd by default —
        # SBUF-only int/f32 ALU work). Keeps Sqrt off ACT (not in the
        # tanh/exp table set) without 40 per-tile call sites.
        # inv_w set: stats arrived as raw [sum, sumsq] — convert to
        # [mean, var] first. inv_w None: bn_aggr already wrote [mean, var].
        eng = eng or nc.vector   # TensorScalarPtr is not legal on Pool
        mu = st[:, lo:lo + n, 0:1]
        va = st[:, lo:lo + n, 1:2]
        if inv_w is not None:
            eng.tensor_scalar_mul(out=mu, in0=mu, scalar1=inv_w)
            msq = pool.tile([P, n], f32, tag=nm + "msq", name=nm + "msq")
            eng.tensor_tensor(out=msq[:], in0=mu, in1=mu, op=Alu.mult)
            eng.tensor_scalar(out=va, in0=va, scalar1=inv_w,
                              scalar2=None, op0=Alu.mult)
            eng.tensor_tensor(out=va, in0=va, in1=msq[:],
                              op=Alu.subtract)
        eng.tensor_scalar_add(out=va, in0=va, scalar1=EPS)
        yi = pool.tile([P, n], i32, tag=nm + "yi", name=nm + "yi")
        eng.tensor_scalar(out=yi[:], in0=va.bitcast(i32),
                          scalar1=1, scalar2=None,
                          op0=Alu.arith_shift_right)
        eng.tensor_scalar(out=yi[:], in0=yi[:], scalar1=-1,
                          scalar2=0x5F3759DF, op0=Alu.mult,
                          op1=Alu.add)
        y = yi[:].bitcast(f32)
        t2 = pool.tile([P, n], f32, tag=nm + "t2", name=nm + "t2")
        for _ in range(2):
            eng.tensor_tensor(out=t2[:], in0=y, in1=y, op=Alu.mult)
            eng.tensor_tensor(out=t2[:], in0=t2[:], in1=va, op=Alu.mult)
            eng.tensor_scalar(out=t2[:], in0=t2[:], scalar1=-0.5,
                              scalar2=1.5, op0=Alu.mult, op1=Alu.add)
            eng.tensor_tensor(out=y, in0=y, in1=t2[:], op=Alu.mult)
        eng.tensor_copy(out=va, in_=y)

    # phase-2 pools created early so vemb group 0 can prefetch in phase 1
    vembp = ctx.enter_context(tc.tile_pool(name="vemb", bufs=2))
    vwork = ctx.enter_context(tc.tile_pool(name="vwork", bufs=2))
    attp = ctx.enter_context(tc.tile_pool(name="att", bufs=2))
    outsp = ctx.enter_context(tc.tile_pool(name="outs", bufs=2))
    vstate = {"v2_prev": None, "vemb1": None, "vemb2": None}

    def vemb_fetch(g):
        vstate["vemb1"] = vembp.tile([P, 4, C], DT, tag="vemb1",
                                     name=f"vemb1_{g}")
        nc.sync.dma_start(vstate["vemb1"][:],
                          vembr1[:, g * 4:(g + 1) * 4, :])
        vstate["vemb2"] = vembp.tile([P, 4, C], DT, tag="vemb2",
                                     name=f"vemb2_{g}")
        nc.sync.dma_start(vstate["vemb2"][:],
                          vembr2[:, g * 4:(g + 1) * 4, :])

    # attention score pools opened for the whole kernel so slot-0/1 score
    # work can fill the PE lull at the end of phase 1 (2 PSUM banks)
    ps_sc = ctx.enter_context(tc.tile_pool(name="ps_sc", bufs=1,
                                           space="PSUM"))
    ps_sum = ctx.enter_context(tc.tile_pool(name="ps_sum", bufs=1,
                                            space="PSUM"))
    astate = {}
    apools = {}
    pending = []                # FIFO of attention emission thunks

    def emit_items(n):
        for _ in range(n):
            if not pending:
                return
            pending.pop(0)()

    # L1(s, kp): scores + tanh + mask + exp + denominator-sum matmuls.
    # Only needs kT/qT tiles < R[s] — runs as PE filler from late phase 1.
    # All slots share one [P, 8] sums psum tile (per-slot column pairs).
    def attn_l1(s, kp, st):
        if kp == 0:
            st[f"ee{s}"] = attp.tile([P, NT, CHUNK], DT, tag="ee",
                                     name=f"ee_{s}", bufs=4)
        if kp == 0:
            # two [P,1] psum tiles (one bank each), reused slot after slot:
            # the next slot's accumulation start waits on this slot's recip
            st["sums"] = [ps_sum.tile([P, 1], f32, tag=f"sums{i}", bufs=1,
                                      name=f"sums_{s}_{i}")
                          for i in range(2)]
        ee = st[f"ee{s}"]
        sps = ps_sc.tile([P, 2, CHUNK], f32, tag="sps", bufs=1,
                         name=f"sps_{s}_{kp}")
        for h in range(2):
            kt = 2 * kp + h
            for qc in range(2):
                nc.tensor.matmul(
                    sps[:, h, :], kT[:, qc, kt * P:(kt + 1) * P],
                    qT[:, qc, s * CHUNK:(s + 1) * CHUNK],
                    start=(qc == 0), stop=(qc == 1))
        et = attp.tile([P, 2, CHUNK], DT, tag="et", name=f"et_{s}_{kp}")
        nc.scalar.activation(et[:], sps[:], Act.Tanh,
                             scale=1.0 / SCORE_SCALE)
        if (s, 2 * kp) in MASK_IDX:
            mi = MASK_IDX[(s, 2 * kp)]
            nc.gpsimd.tensor_tensor(
                out=et[:], in0=et[:],
                in1=maskall[:, mi:mi + 2, :], op=Alu.add)
        nc.scalar.activation(ee[:, 2 * kp:2 * kp + 2, :], et[:], Act.Exp,
                             scale=CAP_SCALE)
        # denominator sums: each (slot, i) column is its own psum TILE —
        # psum allows only one pending accumulation group per tile/region
        for h in range(2):
            kt = 2 * kp + h
            first, last = kt == 0, kt == R[s] - 1
            for i in range(2):
                nc.tensor.matmul(st["sums"][i][:],
                                 ee[:, kt, i * P:(i + 1) * P], ones1[:],
                                 start=first, stop=last)

    def attn_recip(s, st):
        recip = attp.tile([P, 2], f32, tag="recip", name=f"recip_{s}")
        st[f"recip{s}"] = recip
        for i in range(2):
            nc.vector.reciprocal(recip[:, i:i + 1], st["sums"][i][:])

    def add_l1(s):
        for kp in range(R[s] // 2):
            pending.append(lambda s=s, kp=kp: attn_l1(s, kp, astate))
        pending.append(lambda s=s: attn_recip(s, astate))

    # =============== Phase 1: x projections + k/q chains ===============
    # Interleaved per 512-token block: xt DMA -> kv_mid + q matmuls -> k/q
    # tile blend + stats. LN apply / transpose is deferred to rsqrt epochs
    # (after tiles 0-7 and 8-15) so stats batch and slots unlock early.
    shp = ctx.enter_context(tc.tile_pool(name="shp", bufs=4))

    def k_tile(tt, embp, ps_kq):
        g, j = tt // 4, tt % 4
        if j == 0:
            embp["kemb"] = embp["pool"].tile([P, 4, QD], DT, tag="kemb",
                                             name=f"kemb_{g}")
            nc.sync.dma_start(embp["kemb"][:],
                              kembr[:, g * 4:(g + 1) * 4, :])
        kps = ps_kq.tile([P, QD], f32, tag="kqps", bufs=1, name=f"kps{tt}")
        nc.tensor.matmul(kps[:], kvmid[0:KV, tt * P:(tt + 1) * P],
                         wkup[:], start=True, stop=True)
        nc.vector.tensor_tensor(out=kk[:, tt, :], in0=kps[:],
                                in1=embp["kemb"][:, j, :], op=Alu.mult)
        # shift: rows 1..127 via superdiagonal matmul; row 0 seeded with the
        # previous tile's last row by a 1-row gpsimd copy (base 127 -> 0).
        shps = ps_kq.tile([P, QD], f32, tag="kqshps", bufs=2,
                          name=f"kshps{tt}")
        nc.tensor.matmul(shps[:], ssup[:], kk[:, tt, :],
                         start=True, stop=(tt == 0))
        if tt > 0:
            nc.tensor.matmul(shps[:], bnd[:], kk[:, tt - 1, :],
                             start=False, stop=True)
        t1 = shp.tile([P, QD], DT, tag="kt1", name=f"kt1_{tt}")
        nc.vector.tensor_tensor(out=t1[:], in0=shps[:], in1=kk[:, tt, :],
                                op=Alu.subtract)
        nc.gpsimd.tensor_tensor(out=t1[:], in0=t1[:], in1=xk_rep[:],
                                op=Alu.mult)
        nc.gpsimd.tensor_tensor(out=kf[:, tt, :], in0=kk[:, tt, :],
                                in1=t1[:], op=Alu.add)
        st6 = shp.tile([P, 6], f32, tag="kst6", name=f"kst6_{tt}")
        nc.vector.bn_stats(out=st6[:], in_=kf[:, tt, :])
        nc.vector.bn_aggr(out=kst[:, tt, :], in_=st6[:])

    def q_tile(tt, ps_kq):
        s = tt // 2
        qshps = ps_kq.tile([P, QD], f32, tag="kqshps", bufs=2,
                           name=f"qshps{tt}")
        nc.tensor.matmul(qshps[:], ssup[:], qraw[:, tt, :],
                         start=True, stop=False)
        if tt % 2 == 0:
            nc.tensor.matmul(qshps[:], qsel[s][:], qprev[:],
                             start=False, stop=True)
        else:
            nc.tensor.matmul(qshps[:], bnd[:], qraw[:, tt - 1, :],
                             start=False, stop=True)
        t1 = shp.tile([P, QD], DT, tag="qt1", name=f"qt1_{tt}")
        nc.vector.tensor_tensor(out=t1[:], in0=qshps[:],
                                in1=qraw[:, tt, :], op=Alu.subtract)
        nc.gpsimd.tensor_tensor(out=t1[:], in0=t1[:], in1=xq_rep[:],
                                op=Alu.mult)
        nc.gpsimd.tensor_tensor(out=qf[:, tt, :], in0=qraw[:, tt, :],
                                in1=t1[:], op=Alu.add)
        st6 = shp.tile([P, 6], f32, tag="qst6", name=f"qst6_{tt}")
        nc.vector.bn_stats(out=st6[:], in_=qf[:, tt, :])
        nc.vector.bn_aggr(out=qst[:, tt, :], in_=st6[:])

    def kq_epoch(ep, ps_kq):
        # tiles [8*ep, 8*ep+8): batched rsqrt, then LN apply + transpose.
        lo = 8 * ep
        batch_rsqrt(kst, lo, 8, shp, f"krs{ep}")
        if ep == 0:
            batch_rsqrt(qst, 0, 8, shp, "qrs")
        for tt in range(lo, lo + 8):
            nc.vector.tensor_scalar(out=kf[:, tt, :], in0=kf[:, tt, :],
                                    scalar1=kst[:, tt, 0:1],
                                    scalar2=kst[:, tt, 1:2],
                                    op0=Alu.subtract, op1=Alu.mult)
            if gk is not None:
                nc.gpsimd.tensor_tensor(out=kf[:, tt, :], in0=kf[:, tt, :],
                                        in1=gk[:], op=Alu.mult)
                nc.gpsimd.tensor_tensor(out=kf[:, tt, :], in0=kf[:, tt, :],
                                        in1=bk[:], op=Alu.add)
            if ep == 0 and tt < NQT:
                nc.vector.tensor_scalar(out=qf[:, tt, :], in0=qf[:, tt, :],
                                        scalar1=qst[:, tt, 0:1],
                                        scalar2=qst[:, tt, 1:2],
                                        op0=Alu.subtract, op1=Alu.mult)
                if gq is not None:
                    nc.gpsimd.tensor_tensor(out=qf[:, tt, :],
                                            in0=qf[:, tt, :],
                                            in1=gq[:], op=Alu.mult)
                    nc.gpsimd.tensor_tensor(out=qf[:, tt, :],
                                            in0=qf[:, tt, :],
                                            in1=bq[:], op=Alu.add)
        for tt in range(lo, lo + 8):
            tps = ps_kq.tile([P, 2, P], DT, tag="tps", bufs=1,
                             name=f"tpsk{tt}")
            for qc in range(2):
                nc.tensor.transpose(tps[:, qc, :],
                                    kf[:, tt, qc * P:(qc + 1) * P],
                                    ident[:])
            nc.vector.tensor_copy(out=kT[:, :, tt * P:(tt + 1) * P],
                                  in_=tps[:])
            if ep == 0 and tt < NQT:
                tps2 = ps_kq.tile([P, 2, P], DT, tag="tps", bufs=1,
                                  name=f"tpsq{tt}")
                for qc in range(2):
                    nc.tensor.transpose(tps2[:, qc, :],
                                        qf[:, tt, qc * P:(qc + 1) * P],
                                        ident[:])
                nc.vector.tensor_copy(
                    out=qT[:, :, tt * P:(tt + 1) * P], in_=tps2[:])

    with (tc.tile_pool(name="xin", bufs=3) as xin,
          tc.tile_pool(name="emb", bufs=2) as embpool,
          tc.tile_pool(name="ps_a", bufs=2, space="PSUM") as ps_a,
          tc.tile_pool(name="ps_kq", bufs=1, space="PSUM") as ps_kq):
        embp = {"pool": embpool}
        for tb in range(T // 512):
            xt = xin.tile([P, 8, 512], DT, tag="xt", name=f"xt{tb}")
            if tb == 0:
                # split the first block across both HWDGE queues so the
                # leading kv matmuls start ~1.5us sooner
                nc.sync.dma_start(xt[:, 0:4, :],
                                  xTr[:, 0:4, tb * 512:(tb + 1) * 512])
                nc.scalar.dma_start(xt[:, 4:8, :],
                                    xTr[:, 4:8, tb * 512:(tb + 1) * 512])
            else:
                nc.sync.dma_start(xt[:], xTr[:, :, tb * 512:(tb + 1) * 512])
            kvps = ps_a.tile([64, 512], f32, tag="kvps", bufs=1)
            for cc in range(8):
                nc.tensor.matmul(kvps[:], wkv[:, cc, :], xt[:, cc, :],
                                 start=(cc == 0), stop=(cc == 7))
            nc.scalar.copy(out=kvmid[:, tb * 512:(tb + 1) * 512],
                           in_=kvps[:])
            if tb < 2:
                xqt = xin.tile([P, 8, 512], DT, tag="xt", name=f"xqt{tb}")
                nc.scalar.dma_start(xqt[:],
                                    xqTr[:, :, tb * 512:(tb + 1) * 512])
                for j in range(4):
                    tt = tb * 4 + j
                    qps = ps_kq.tile([P, QD], f32, tag="kqps", bufs=1,
                                     name=f"qps{tt}")
                    for cc in range(8):
                        nc.tensor.matmul(qps[:],
                                         xqt[:, cc, j * P:(j + 1) * P],
                                         wqq[:, cc, :],
                                         start=(cc == 0), stop=(cc == 7))
                    nc.scalar.copy(out=qraw[:, tt, :], in_=qps[:])
            if tb == 0:
                xqp = xin.tile([P, 8, NSLOT], DT, tag="xqp")
                nc.sync.dma_start(xqp[:], xqpr[:])
                qpps = ps_kq.tile([NSLOT, QD], f32, tag="kqps", bufs=1,
                                  name="qpps")
                for cc in range(8):
                    nc.tensor.matmul(qpps[:], xqp[:, cc, :], wqq[:, cc, :],
                                     start=(cc == 0), stop=(cc == 7))
                nc.scalar.copy(out=qprev[:], in_=qpps[:])
            for tt in range(4 * tb, 4 * tb + 4):
                k_tile(tt, embp, ps_kq)
                if tt < NQT:
                    q_tile(tt, ps_kq)
                if tb >= 2:
                    emit_items(1)   # slot-0/1 score work as PE filler
            if tb == 1:
                kq_epoch(0, ps_kq)
                pass
                add_l1(0)
            if tb == 2:
                vemb_fetch(0)   # prefetch first v-emb group for phase 2
                add_l1(1)
        kq_epoch(1, ps_kq)

    if phases < 2:
        if loop is not None:
            loop.__exit__(None, None, None)
        ctx.close()
        return

    # =============== Phase 2: v chain interleaved with attention ========
    def emit_v(tt, ps_v, vembs):
        j = tt % 4
        vemb1, vemb2 = vembs
        v2_prev = vstate["v2_prev"]
        # vps and shps rotate through one double-buffered full-width tag
        # (4 banks): vps(t+1) waits only on tanh(t), shps(t) on the blend.
        # half-width psum tiles (1 bank each, double-buffered) keep the
        # v chain pipelined across tiles within 4 banks total.
        vt = vwork.tile([P, C], DT, tag="vt", name=f"vt{tt}")
        for h in range(2):
            vps = ps_v.tile([P, 512], f32, tag="vps", bufs=2,
                            name=f"vps{tt}_{h}")
            nc.tensor.matmul(vps[:],
                             kvmid[KV:64, tt * P:(tt + 1) * P],
                             wvup[:, h * 512:(h + 1) * 512],
                             start=True, stop=True)
            nc.scalar.activation(vt[:, h * 512:(h + 1) * 512], vps[:],
                                 Act.Tanh)
        v2 = vwork.tile([P, C], DT, tag="v2", name=f"v2_{tt}")
        nc.gpsimd.tensor_tensor(out=vv[:, tt, :], in0=vt[:],
                                in1=vemb1[:, j, :], op=Alu.mult)
        nc.gpsimd.tensor_tensor(out=v2[:], in0=vt[:],
                                in1=vemb2[:, j, :], op=Alu.mult)
        for h in range(2):
            cs = slice(h * 512, (h + 1) * 512)
            shps = ps_v.tile([P, 512], f32, tag="vsh", bufs=1,
                             name=f"vsh{tt}_{h}")
            nc.tensor.matmul(shps[:], ssup[:], v2[:, cs], start=True,
                             stop=v2_prev is None)
            if v2_prev is not None:
                nc.tensor.matmul(shps[:], bnd[:], v2_prev[:, cs],
                                 start=False, stop=True)
            # blend add + half row-sum in one DVE pass; sumsq in one more
            # (vt is dead here — reuse as the unread elementwise output;
            # sumsq uses scalar_tensor_tensor since tensor_tensor_reduce
            # hangs the DVE on hardware).
            nc.vector.scalar_tensor_tensor(
                out=vv[:, tt, cs], in0=shps[:], scalar=1.0,
                in1=vv[:, tt, cs],
                op0=Alu.mult, op1=Alu.add, accum_out=vst[:, tt, h:h + 1])
            nc.vector.scalar_tensor_tensor(
                out=vt[:, cs], in0=vv[:, tt, cs], scalar=1.0,
                in1=vv[:, tt, cs],
                op0=Alu.mult, op1=Alu.mult,
                accum_out=vst[:, tt, 2 + h:3 + h])
        vstate["v2_prev"] = v2

    def v_group_ln(g):
        # tiles [4g, 4g+4): merge half-sums, batched rsqrt, then LN apply
        # split across Pool and DVE halves (halves the group bubble).
        s4 = slice(4 * g, 4 * g + 4)
        nc.vector.tensor_tensor(out=vst[:, s4, 0:1], in0=vst[:, s4, 0:1],
                                in1=vst[:, s4, 1:2], op=Alu.add)
        nc.vector.tensor_tensor(out=vst[:, s4, 1:2], in0=vst[:, s4, 2:3],
                                in1=vst[:, s4, 3:4], op=Alu.add)
        batch_rsqrt(vst, 4 * g, 4, vwork, f"vrs{g}", inv_w=1.0 / C)
        for tt in range(4 * g, 4 * g + 4):
            nc.vector.tensor_scalar(out=vv[:, tt, :], in0=vv[:, tt, :],
                                    scalar1=vst[:, tt, 0:1],
                                    scalar2=vst[:, tt, 1:2],
                                    op0=Alu.subtract, op1=Alu.mult)
            if gv is not None:
                nc.gpsimd.tensor_tensor(out=vv[:, tt, :], in0=vv[:, tt, :],
                                        in1=gv[:], op=Alu.mult)
                nc.gpsimd.tensor_tensor(out=vv[:, tt, :], in0=vv[:, tt, :],
                                        in1=bv[:], op=Alu.add)

    # L2(s, ch, kp): out-psum accumulation for C-half ch (replays the ee
    # strip) — needs LN'd v tiles < R[s]. 2 psum banks total.
    def attn_l2(s, ch, kp, ps_out, st):
        if ch == 0 and kp == 0:
            st["ot"] = [outsp.tile([P, C], DT, tag=f"ot{i}",
                                   name=f"ot_{s}_{i}") for i in range(2)]
        ee, ot, recip = st[f"ee{s}"], st["ot"], st[f"recip{s}"]
        cs = slice(ch * 512, (ch + 1) * 512)
        last_kp = kp == R[s] // 2 - 1
        for h in range(2):
            kt = 2 * kp + h
            for i in range(2):
                opsi = ps_out.tile([P, 512], f32, tag=f"o{i}", bufs=1,
                                   name=f"ops_{s}_{ch}_{i}") \
                    if kp == 0 and h == 0 else st[f"o{i}"]
                if kp == 0 and h == 0:
                    st[f"o{i}"] = opsi
                nc.tensor.matmul(opsi[:], ee[:, kt, i * P:(i + 1) * P],
                                 vv[:, kt, cs],
                                 start=(kt == 0), stop=(kt == R[s] - 1))
        if last_kp:
            for i in range(2):
                nc.vector.tensor_scalar_mul(
                    out=ot[i][:, cs], in0=st[f"o{i}"][:],
                    scalar1=recip[:, i:i + 1])
            if ch == 1:
                for i in range(2):
                    nc.gpsimd.dma_start(
                        out_d[s * CHUNK + i * P:s * CHUNK + (i + 1) * P, :],
                        ot[i][:])

    with (tc.tile_pool(name="ps_v", bufs=1, space="PSUM") as ps_v,
          tc.tile_pool(name="ps_out", bufs=1, space="PSUM") as ps_out):

        def add_l2(s):
            for ch in range(2):
                for kp in range(R[s] // 2):
                    pending.append(
                        lambda s=s, ch=ch, kp=kp:
                        attn_l2(s, ch, kp, ps_out, astate))

        # slots 2/3 score work (0/1 were queued during phase 1)
        add_l1(2)
        add_l1(3)
        for tt in range(NT):
            g, j = tt // 4, tt % 4
            if j == 0:
                vembs = (vstate["vemb1"], vstate["vemb2"])
                if g < 3:
                    vemb_fetch(g + 1)   # prefetch next group
            emit_v(tt, ps_v, vembs)
            if j == 3:
                v_group_ln(g)
                add_l2(g)
            emit_items(3 if tt < 4 else 5)
        emit_items(len(pending))

    if loop is not None:
        loop.__exit__(None, None, None)
    ctx.close()


_NC_CACHE = {}


def _input_specs(apply_gb, bf16):
    import concourse.mybir as mybir
    f32 = mybir.dt.float32
    DT = mybir.dt.bfloat16 if bf16 else f32
    specs = [
        ("xT", [C, T], DT), ("xqT", [C, TQ], DT),
        ("xqprevT", [C, NSLOT], DT),
        ("kemb", [T, QD], DT), ("vemb1", [T, C], DT),
        ("vemb2", [T, C], DT),
        ("wqq", [C, QD], DT), ("wkv", [C, 64], DT),
        ("wkup", [KV, QD], DT), ("wvup", [KV, C], DT),
        ("xq_rep", [P, QD], DT), ("xk_rep", [P, QD], DT),
        ("mask", [NMASK, P, CHUNK], DT),
    ]
    if apply_gb:
        specs += [("gq_rep", [P, QD], DT), ("bq_rep", [P, QD], DT),
                  ("gk_rep", [P, QD], DT), ("bk_rep", [P, QD], DT),
                  ("gv_rep", [P, C], DT), ("bv_rep", [P, C], DT)]
    return specs


def get_nc(apply_gb, bf16=True, nrep=1, phases=4):
    key = (bool(apply_gb), bool(bf16), int(nrep), int(phases))
    if key in _NC_CACHE:
        return _NC_CACHE[key]
    import concourse.mybir as mybir
    import concourse.tile as tile
    from concourse import bacc

    nc = bacc.Bacc("TRN2", target_bir_lowering=False, debug=False,
                   num_devices=N_CORES)
    a = {}
    for name, shape, dt in _input_specs(apply_gb, bf16):
        a[name] = nc.dram_tensor(name, shape, dt, kind="ExternalInput").ap()
    DT = mybir.dt.bfloat16 if bf16 else mybir.dt.float32
    a["out"] = nc.dram_tensor("out", [TQ, C], DT,
                              kind="ExternalOutput").ap()
    with tile.TileContext(nc) as tc:
        _build_program(nc, tc, a, apply_gb, bf16, nrep=nrep, phases=phases)
    nc.compile()
    _NC_CACHE[key] = nc
    return nc


def _parity_mask(parity):
    m = np.zeros((NMASK, P, CHUNK), np.float32)
    for (s, kt), mi in MASK_IDX.items():
        qs = CHUNKS[parity][s] * CHUNK
        kg = np.arange(P, dtype=np.int64)[:, None] + P * kt
        qg = np.arange(CHUNK, dtype=np.int64)[None, :] + qs
        m[mi] = np.where(qg >= kg, 0.0, NEG).astype(np.float32)
    return m


def make_in_maps(inputs, bf16=True):
    import ml_dtypes
    cdt = ml_dtypes.bfloat16 if bf16 else np.float32

    x = np.asarray(inputs["x"], np.float32)
    idx = np.asarray(inputs["idx"]).astype(np.int64)
    k_tab = np.asarray(inputs["k_emb_tab"], np.float32)
    v_tab = np.asarray(inputs["v_emb_tab"], np.float32)
    W_qq = np.asarray(inputs["W_qq"], np.float32)
    W_k = np.asarray(inputs["W_k"], np.float32)
    W_kup = np.asarray(inputs["W_kup"], np.float32)
    W_v = np.asarray(inputs["W_v"], np.float32)
    W_vup = np.asarray(inputs["W_vup"], np.float32)
    x_q = np.asarray(inputs["x_q"], np.float32).reshape(QD)
    x_k = np.asarray(inputs["x_k"], np.float32).reshape(QD)
    x_v = np.asarray(inputs["x_v"], np.float32).reshape(C)
    g_q = np.asarray(inputs["g_q"], np.float32).reshape(QD)
    b_q = np.asarray(inputs["b_q"], np.float32).reshape(QD)
    g_k = np.asarray(inputs["g_k"], np.float32).reshape(QD)
    b_k = np.asarray(inputs["b_k"], np.float32).reshape(QD)
    g_v = np.asarray(inputs["g_v"], np.float32).reshape(C)
    b_v = np.asarray(inputs["b_v"], np.float32).reshape(C)

    apply_gb = not (np.all(g_q == 1) and np.all(b_q == 0)
                    and np.all(g_k == 1) and np.all(b_k == 0)
                    and np.all(g_v == 1) and np.all(b_v == 0))

    k_emb = k_tab[idx]          # [B, T, QD]
    v_emb = v_tab[idx]          # [B, T, C]
    vemb1 = [np.ascontiguousarray(v_emb[b] * (1.0 - x_v)).astype(cdt)
             for b in range(B)]
    vemb2 = [np.ascontiguousarray(v_emb[b] * x_v).astype(cdt)
             for b in range(B)]

    def cvt(arr):
        return np.ascontiguousarray(arr).astype(cdt)

    shared = {
        "wqq": cvt(W_qq.T),
        "wkv": cvt(np.concatenate([W_k, W_v], 0).T),
        "wkup": cvt(W_kup.T),
        "wvup": cvt(W_vup.T),
        "xq_rep": cvt(np.broadcast_to(x_q, (P, QD))),
        "xk_rep": cvt(np.broadcast_to(x_k, (P, QD))),
    }
    if apply_gb:
        for nm, v in [("gq", g_q), ("bq", b_q), ("gk", g_k), ("bk", b_k)]:
            shared[nm + "_rep"] = cvt(np.broadcast_to(v, (P, QD)))
        for nm, v in [("gv", g_v), ("bv", b_v)]:
            shared[nm + "_rep"] = cvt(np.broadcast_to(v, (P, C)))

    pmask = [_parity_mask(0).astype(cdt), _parity_mask(1).astype(cdt)]
    in_maps = []
    for c in range(N_CORES):
        b, parity = c // 2, c % 2
        chunks = CHUNKS[parity]
        cols = np.concatenate([np.arange(ch * CHUNK, (ch + 1) * CHUNK)
                               for ch in chunks])
        xqprev = np.zeros((NSLOT, C), np.float32)
        for j, ch in enumerate(chunks):
            if ch > 0:
                xqprev[j] = x[b, ch * CHUNK - 1]
        m = dict(shared)
        m.update(
            xT=cvt(x[b].T), xqT=cvt(x[b][cols].T),
            xqprevT=cvt(xqprev.T),
            kemb=cvt(k_emb[b]),
            vemb1=vemb1[b], vemb2=vemb2[b],
            mask=pmask[parity],
        )
        in_maps.append(m)
    return in_maps, apply_gb


def assemble_output(results):
    out = np.empty((B, T, C), np.float32)
    for c in range(N_CORES):
        oc = np.asarray(results[c]["out"]).astype(np.float32)
        for j, ch in enumerate(CHUNKS[c % 2]):
            out[c // 2, ch * CHUNK:(ch + 1) * CHUNK] = \
                oc[j * CHUNK:(j + 1) * CHUNK]
    return out


BF16 = True


def kernel(**inputs):
    from concourse.bass_utils import run_bass_kernel_spmd
    in_maps, apply_gb = make_in_maps(inputs, bf16=BF16)
    nc = get_nc(apply_gb, bf16=BF16)
    res = run_bass_kernel_spmd(nc, in_maps, core_ids=list(range(N_CORES)))
    return assemble_output(res.results)


# revision 89
# speedup vs baseline: 1.0348x; 1.0348x over previous
"""DeepEmbedAttention TRN2 kernel — 8-core SPMD, v2.

Sharding: 2 cores per batch (B=4). Each core computes the full k/v chain for
its batch (T=2048) and attention outputs for 4 query chunks of 256 tokens.
Chunk assignment is causally load-balanced: even cores take chunks {0,3,4,7},
odd cores {1,2,5,6}. The single SPMD program processes chunks at canonical
slot positions; everything position-dependent (q columns, causal masks,
chunk-boundary tokens) arrives as per-core input data, so one program serves
all 8 cores. Softmax needs no max-subtraction: scores are tanh-capped to
[-64, 64], so exp() cannot overflow fp32.

v2 engine plan (from sim-profile of v1: ACT 73% busy was the bottleneck,
with ~40us of DMA issue + ~31us of mean/var accum passes riding the scalar
queue):
 - All activations stay inside the one co-resident ACT table set
   (exp_and_others: tanh/exp/square/copy) — single table load.
 - v-chain layernorm stats fused into DVE passes: blend-add carries the
   row-sum (scalar_tensor_tensor accum_out), sum-of-squares is one
   tensor_tensor_reduce pass. Nothing on ACT but the tanh.
 - token-shift: superdiagonal PE matmul for rows 1..127; row 0 (the carry
   from the previous tile) is seeded into PSUM by a 1-row gpsimd copy with
   mismatched partition bases instead of a full-width boundary matmul.
 - rsqrt for layernorm runs as batched magic-rsqrt epochs (groups of tiles)
   instead of 40 per-tile call sites.
 - Attention is interleaved with the v chain: slot s is emitted as soon as
   v tiles < 4(s+1) exist. PSUM budget: v-chain 2 banks (single-buffered
   pool holds vps then vshps) + scores 1 + sums 1 + out 4 = 8.
 - DMA issue is spread across the three issuing queues (sync / scalar /
   gpsimd); output is stored bf16 and widened on host.
"""

import sys

if "/opt/trn_rl_repo" not in sys.path:
    sys.path.insert(0, "/opt/trn_rl_repo")

import numpy as np

B, T, C = 4, 2048, 1024
QD, KV = 256, 32
SCORE_SCALE, CAP_SCALE = 1024.0, 64.0
EPS = 1e-5
N_CORES = 8
P = 128
CHUNK = 256
NSLOT = 4                       # q-chunks per core
TQ = NSLOT * CHUNK              # 1024 canonical query tokens per core
NT = T // P                     # 16 token tiles (full sequence)
NQT = TQ // P                   # 8 canonical query token tiles
CHUNKS = [[0, 3, 4, 7], [1, 2, 5, 6]]   # parity -> global chunk ids
R = [4, 8, 12, 16]              # k-tiles per slot (max over parities)
MINQS = [0, 512, 1024, 1536]    # min chunk start over parities, per slot
NEED_MASK = [(s, kt) for s in range(NSLOT) for kt in range(R[s])
             if P * (kt + 1) > MINQS[s]]
MASK_IDX = {sk: i for i, sk in enumerate(NEED_MASK)}
NMASK = len(NEED_MASK)          # 16
NEG = -1.0e30


def _build_program(nc, tc, a, apply_gb, bf16, nrep=1, phases=4):
    from contextlib import ExitStack

    import concourse.mybir as mybir
    from concourse.masks import make_identity

    f32 = mybir.dt.float32
    i32 = mybir.dt.int32
    DT = mybir.dt.bfloat16 if bf16 else f32
    Alu = mybir.AluOpType
    Act = mybir.ActivationFunctionType

    xTr = a["xT"].rearrange("(a p) t -> p a t", p=P)        # [128, 8, 2048]
    xqTr = a["xqT"].rearrange("(a p) t -> p a t", p=P)      # [128, 8, 1024]
    xqpr = a["xqprevT"].rearrange("(a p) t -> p a t", p=P)  # [128, 8, 4]
    wqqr = a["wqq"].rearrange("(a p) d -> p a d", p=P)      # [128, 8, 256]
    wkvr = a["wkv"].rearrange("(a p) d -> p a d", p=P)      # [128, 8, 64]
    kembr = a["kemb"].rearrange("(g p) d -> p g d", p=P)    # [128, 16, 256]
    vembr1 = a["vemb1"].rearrange("(g p) d -> p g d", p=P)  # [128, 16, 1024]
    vembr2 = a["vemb2"].rearrange("(g p) d -> p g d", p=P)
    maskr = a["mask"].rearrange("m p q -> p m q")           # [128, 16, 256]
    out_d = a["out"]                                        # [1024, 1024] bf16

    ctx = ExitStack()
    const = ctx.enter_context(tc.tile_pool(name="const", bufs=1))
    pers = ctx.enter_context(tc.tile_pool(name="pers", bufs=1))

    # --- constants (gpsimd queue for the DMAs) ---
    ident = const.tile([P, P], DT, tag="ident")
    make_identity(nc, ident[:])
    # ssup[p, m] = 1 iff m == p+1 : shift-down-one (sh[m] = v[m-1]), row 0 = 0
    ssup = const.tile([P, P], DT, tag="ssup")
    nc.gpsimd.memset(ssup[:], 0.0)
    nc.gpsimd.affine_select(out=ssup[:], in_=ssup[:],
                            compare_op=Alu.not_equal, fill=1.0,
                            base=1, pattern=[[-1, P]], channel_multiplier=1)
    # bnd[p, m] = 1 iff (p==127, m==0) : carry prev tile's last row into row 0
    bnd = const.tile([P, P], DT, tag="bnd")
    nc.gpsimd.memset(bnd[:], 0.0)
    nc.gpsimd.affine_select(out=bnd[:], in_=bnd[:],
                            compare_op=Alu.not_equal, fill=1.0,
                            base=-(P - 1), pattern=[[-P, P]],
                            channel_multiplier=1)
    # qsel[s][p, m] = 1 iff (p==s, m==0) : qprev row s into row 0
    qsel = []
    for s in range(NSLOT):
        qs_t = const.tile([NSLOT, P], DT, tag=f"qsel{s}", name=f"qsel{s}")
        nc.gpsimd.memset(qs_t[:], 0.0)
        nc.gpsimd.affine_select(out=qs_t[:], in_=qs_t[:],
                                compare_op=Alu.not_equal, fill=1.0,
                                base=-s, pattern=[[-NSLOT, P]],
                                channel_multiplier=1)
        qsel.append(qs_t)
    ones1 = const.tile([P, 1], DT, tag="ones1")
    nc.gpsimd.memset(ones1[:], 1.0)

    # weights that gate phase-1 compute go first, on the fast HWDGE queues
    wkv = const.tile([P, 8, 64], DT, tag="wkv")
    nc.sync.dma_start(wkv[:], wkvr[:])
    wqq = const.tile([P, 8, QD], DT, tag="wqq")
    nc.scalar.dma_start(wqq[:], wqqr[:])
    wkup = const.tile([KV, QD], DT, tag="wkup")
    nc.gpsimd.dma_start(wkup[:], a["wkup"][:])
    # v_mid lives at base partition 32 inside kvmid; PE needs lhsT/rhs bases
    # to match, so W_vupT is loaded at partitions 32..63 as well.
    wvup64 = const.tile([64, C], DT, tag="wvup")
    nc.gpsimd.dma_start(wvup64[KV:64, :], a["wvup"][:])
    wvup = wvup64[KV:64, :]
    xq_rep = const.tile([P, QD], DT, tag="xq_rep")
    nc.gpsimd.dma_start(xq_rep[:], a["xq_rep"][:])
    xk_rep = const.tile([P, QD], DT, tag="xk_rep")
    nc.gpsimd.dma_start(xk_rep[:], a["xk_rep"][:])
    maskall = const.tile([P, NMASK, CHUNK], DT, tag="maskall")
    nc.gpsimd.dma_start(maskall[:], maskr[:])
    gb = {}
    if apply_gb:
        for nm, d in [("gq", QD), ("bq", QD), ("gk", QD), ("bk", QD),
                      ("gv", C), ("bv", C)]:
            gb[nm] = const.tile([P, d], DT, tag=nm + "_rep", name=nm + "_rep")
            nc.gpsimd.dma_start(gb[nm][:], a[nm + "_rep"][:])

    loop = tc.For_i(0, nrep, 1) if nrep > 1 else None
    if loop is not None:
        loop.__enter__()

    # --- persistent strips ---
    kvmid = pers.tile([64, T], DT, tag="kvmid")       # [k_mid; v_mid]^T
    qraw = pers.tile([P, NQT, QD], DT, tag="qraw")    # canonical q tiles
    qprev = pers.tile([NSLOT, QD], DT, tag="qprev")   # chunk-boundary q rows
    kk = pers.tile([P, NT, QD], DT, tag="kk")         # pre-blend k
    kf = pers.tile([P, NT, QD], DT, tag="kf")         # post-LN k
    qf = pers.tile([P, NQT, QD], DT, tag="qf")        # post-LN q
    vv = pers.tile([P, NT, C], DT, tag="vv")          # post-LN v
    kT = pers.tile([P, 2, T], DT, tag="kT")           # k^T for attention
    qT = pers.tile([P, 2, TQ], DT, tag="qT")          # q^T for attention
    # layernorm stats strips: [sum; sumsq] per tile, finished in rsqrt epochs
    kst = pers.tile([P, NT, 2], f32, tag="kst")
    qst = pers.tile([P, NQT, 2], f32, tag="qst")
    # v stats: [sum_h0, sum_h1, ssq_h0, ssq_h1] -> [bias, rstd, _, _]
    vst = pers.tile([P, NT, 4], f32, tag="vst")

    gq, bq = (gb.get("gq"), gb.get("bq"))
    gk, bk = (gb.get("gk"), gb.get("bk"))
    gv, bv = (gb.get("gv"), gb.get("bv"))

    def batch_rsqrt(st, lo, n, pool, nm, inv_w=None, eng=None):
        # st[:, lo:lo+n, 1] <- rsqrt(var + EPS) via magic-constant seed + 2
        # Newton iterations, batched over n tiles (on GpSimd by default —
        # SBUF-only int/f32 ALU work). Keeps Sqrt off ACT (not in the
        # tanh/exp table set) without 40 per-tile call sites.
        # inv_w set: stats arrived as raw [sum, sumsq] — convert to
        # [mean, var] first. inv_w None: bn_aggr already wrote [mean, var].
        eng = eng or nc.vector   # TensorScalarPtr is not legal on Pool
        mu = st[:, lo:lo + n, 0:1]
        va = st[:, lo:lo + n, 1:2]
        if inv_w is not None:
            eng.tensor_scalar_mul(out=mu, in0=mu, scalar1=inv_w)
            msq = pool.tile([P, n], f32, tag=nm + "msq", name=nm + "msq")
            eng.tensor_tensor(out=msq[:], in0=mu, in1=mu, op=Alu.mult)
            eng.tensor_scalar(out=va, in0=va, scalar1=inv_w,
                              scalar2=None, op0=Alu.mult)
            eng.tensor_tensor(out=va, in0=va, in1=msq[:],
                              op=Alu.subtract)
        eng.tensor_scalar_add(out=va, in0=va, scalar1=EPS)
        yi = pool.tile([P, n], i32, tag=nm + "yi", name=nm + "yi")
        eng.tensor_scalar(out=yi[:], in0=va.bitcast(i32),
                          scalar1=1, scalar2=None,
                          op0=Alu.arith_shift_right)
        eng.tensor_scalar(out=yi[:], in0=yi[:], scalar1=-1,
                          scalar2=0x5F3759DF, op0=Alu.mult,
                          op1=Alu.add)
        y = yi[:].bitcast(f32)
        t2 = pool.tile([P, n], f32, tag=nm + "t2", name=nm + "t2")
        for _ in range(2):
            eng.tensor_tensor(out=t2[:], in0=y, in1=y, op=Alu.mult)
            eng.tensor_tensor(out=t2[:], in0=t2[:], in1=va, op=Alu.mult)
            eng.tensor_scalar(out=t2[:], in0=t2[:], scalar1=-0.5,
                              scalar2=1.5, op0=Alu.mult, op1=Alu.add)
            eng.tensor_tensor(out=y, in0=y, in1=t2[:], op=Alu.mult)
        eng.tensor_copy(out=va, in_=y)

    # phase-2 pools created early so vemb group 0 can prefetch in phase 1
    vembp = ctx.enter_context(tc.tile_pool(name="vemb", bufs=2))
    vwork = ctx.enter_context(tc.tile_pool(name="vwork", bufs=2))
    attp = ctx.enter_context(tc.tile_pool(name="att", bufs=2))
    outsp = ctx.enter_context(tc.tile_pool(name="outs", bufs=2))
    vstate = {"v2_prev": None, "vemb1": None, "vemb2": None}

    def vemb_fetch(g):
        vstate["vemb1"] = vembp.tile([P, 4, C], DT, tag="vemb1",
                                     name=f"vemb1_{g}")
        nc.sync.dma_start(vstate["vemb1"][:],
                          vembr1[:, g * 4:(g + 1) * 4, :])
        vstate["vemb2"] = vembp.tile([P, 4, C], DT, tag="vemb2",
                                     name=f"vemb2_{g}")
        nc.sync.dma_start(vstate["vemb2"][:],
                          vembr2[:, g * 4:(g + 1) * 4, :])

    astate = {}
    apools = {}
    pending = []                # FIFO of attention emission thunks

    def emit_items(n):
        for _ in range(n):
            if not pending:
                return
            pending.pop(0)()

    # L1(s, kp): scores + tanh + mask + exp into the slot's ee strip.
    def attn_l1(s, kp, st):
        if kp == 0:
            st[f"ee{s}"] = attp.tile([P, NT, CHUNK], DT, tag="ee",
                                     name=f"ee_{s}", bufs=4)
        ee = st[f"ee{s}"]
        sps = apools["ps_sc"].tile([P, 2, CHUNK], f32, tag="sps", bufs=1,
                                   name=f"sps_{s}_{kp}")
        for h in range(2):
            kt = 2 * kp + h
            for qc in range(2):
                nc.tensor.matmul(
                    sps[:, h, :], kT[:, qc, kt * P:(kt + 1) * P],
                    qT[:, qc, s * CHUNK:(s + 1) * CHUNK],
                    start=(qc == 0), stop=(qc == 1))
        et = attp.tile([P, 2, CHUNK], DT, tag="et", name=f"et_{s}_{kp}")
        nc.scalar.activation(et[:], sps[:], Act.Tanh,
                             scale=1.0 / SCORE_SCALE)
        if (s, 2 * kp) in MASK_IDX:
            mi = MASK_IDX[(s, 2 * kp)]
            nc.gpsimd.tensor_tensor(
                out=et[:], in0=et[:],
                in1=maskall[:, mi:mi + 2, :], op=Alu.add)
        nc.scalar.activation(ee[:, 2 * kp:2 * kp + 2, :], et[:], Act.Exp,
                             scale=CAP_SCALE)

    def attn_sums_recip(s, st):
        # denominator sums: ONE shared psum tile; per (slot, i) the
        # accumulation group is contiguous (psum allows a single pending
        # group per tile), recip drains it before the next group starts
        ee = st[f"ee{s}"]
        recip = attp.tile([P, 2], f32, tag="recip", name=f"recip_{s}")
        st[f"recip{s}"] = recip
        for i in range(2):
            sums = apools["ps_sum"].tile([P, 1], f32, tag="sums", bufs=1,
                                         name=f"sums_{s}_{i}")
            for kt in range(R[s]):
                nc.tensor.matmul(sums[:],
                                 ee[:, kt, i * P:(i + 1) * P], ones1[:],
                                 start=(kt == 0), stop=(kt == R[s] - 1))
            nc.vector.reciprocal(recip[:, i:i + 1], sums[:])

    def add_l1(s):
        for kp in range(R[s] // 2):
            pending.append(lambda s=s, kp=kp: attn_l1(s, kp, astate))
        pending.append(lambda s=s: attn_sums_recip(s, astate))

    # =============== Phase 1: x projections + k/q chains ===============
    # Interleaved per 512-token block: xt DMA -> kv_mid + q matmuls -> k/q
    # tile blend + stats. LN apply / transpose is deferred to rsqrt epochs
    # (after tiles 0-7 and 8-15) so stats batch and slots unlock early.
    shp = ctx.enter_context(tc.tile_pool(name="shp", bufs=4))

    def k_tile(tt, embp, ps_kq):
        g, j = tt // 4, tt % 4
        if j == 0:
            embp["kemb"] = embp["pool"].tile([P, 4, QD], DT, tag="kemb",
                                             name=f"kemb_{g}")
            nc.sync.dma_start(embp["kemb"][:],
                              kembr[:, g * 4:(g + 1) * 4, :])
        kps = ps_kq.tile([P, QD], f32, tag="kqps", bufs=2, name=f"kps{tt}")
        nc.tensor.matmul(kps[:], kvmid[0:KV, tt * P:(tt + 1) * P],
                         wkup[:], start=True, stop=True)
        nc.vector.tensor_tensor(out=kk[:, tt, :], in0=kps[:],
                                in1=embp["kemb"][:, j, :], op=Alu.mult)
        # shift: rows 1..127 via superdiagonal matmul; row 0 seeded with the
        # previous tile's last row by a 1-row gpsimd copy (base 127 -> 0).
        shps = ps_kq.tile([P, QD], f32, tag="kqshps", bufs=2,
                          name=f"kshps{tt}")
        nc.tensor.matmul(shps[:], ssup[:], kk[:, tt, :],
                         start=True, stop=(tt == 0))
        if tt > 0:
            nc.tensor.matmul(shps[:], bnd[:], kk[:, tt - 1, :],
                             start=False, stop=True)
        t1 = shp.tile([P, QD], DT, tag="kt1", name=f"kt1_{tt}")
        nc.vector.tensor_tensor(out=t1[:], in0=shps[:], in1=kk[:, tt, :],
                                op=Alu.subtract)
        nc.gpsimd.tensor_tensor(out=t1[:], in0=t1[:], in1=xk_rep[:],
                                op=Alu.mult)
        nc.gpsimd.tensor_tensor(out=kf[:, tt, :], in0=kk[:, tt, :],
                                in1=t1[:], op=Alu.add)
        st6 = shp.tile([P, 6], f32, tag="kst6", name=f"kst6_{tt}")
        nc.vector.bn_stats(out=st6[:], in_=kf[:, tt, :])
        nc.vector.bn_aggr(out=kst[:, tt, :], in_=st6[:])

    def q_tile(tt, ps_kq):
        s = tt // 2
        qshps = ps_kq.tile([P, QD], f32, tag="kqshps", bufs=2,
                           name=f"qshps{tt}")
        nc.tensor.matmul(qshps[:], ssup[:], qraw[:, tt, :],
                         start=True, stop=False)
        if tt % 2 == 0:
            nc.tensor.matmul(qshps[:], qsel[s][:], qprev[:],
                             start=False, stop=True)
        else:
            nc.tensor.matmul(qshps[:], bnd[:], qraw[:, tt - 1, :],
                             start=False, stop=True)
        t1 = shp.tile([P, QD], DT, tag="qt1", name=f"qt1_{tt}")
        nc.vector.tensor_tensor(out=t1[:], in0=qshps[:],
                                in1=qraw[:, tt, :], op=Alu.subtract)
        nc.gpsimd.tensor_tensor(out=t1[:], in0=t1[:], in1=xq_rep[:],
                                op=Alu.mult)
        nc.gpsimd.tensor_tensor(out=qf[:, tt, :], in0=qraw[:, tt, :],
                                in1=t1[:], op=Alu.add)
        st6 = shp.tile([P, 6], f32, tag="qst6", name=f"qst6_{tt}")
        nc.vector.bn_stats(out=st6[:], in_=qf[:, tt, :])
        nc.vector.bn_aggr(out=qst[:, tt, :], in_=st6[:])

    def kq_epoch(ep, ps_kq):
        # tiles [8*ep, 8*ep+8): batched rsqrt, then LN apply + transpose.
        lo = 8 * ep
        batch_rsqrt(kst, lo, 8, shp, f"krs{ep}")
        if ep == 0:
            batch_rsqrt(qst, 0, 8, shp, "qrs")
        for tt in range(lo, lo + 8):
            nc.vector.tensor_scalar(out=kf[:, tt, :], in0=kf[:, tt, :],
                                    scalar1=kst[:, tt, 0:1],
                                    scalar2=kst[:, tt, 1:2],
                                    op0=Alu.subtract, op1=Alu.mult)
            if gk is not None:
                nc.gpsimd.tensor_tensor(out=kf[:, tt, :], in0=kf[:, tt, :],
                                        in1=gk[:], op=Alu.mult)
                nc.gpsimd.tensor_tensor(out=kf[:, tt, :], in0=kf[:, tt, :],
                                        in1=bk[:], op=Alu.add)
            if ep == 0 and tt < NQT:
                nc.vector.tensor_scalar(out=qf[:, tt, :], in0=qf[:, tt, :],
                                        scalar1=qst[:, tt, 0:1],
                                        scalar2=qst[:, tt, 1:2],
                                        op0=Alu.subtract, op1=Alu.mult)
                if gq is not None:
                    nc.gpsimd.tensor_tensor(out=qf[:, tt, :],
                                            in0=qf[:, tt, :],
                                            in1=gq[:], op=Alu.mult)
                    nc.gpsimd.tensor_tensor(out=qf[:, tt, :],
                                            in0=qf[:, tt, :],
                                            in1=bq[:], op=Alu.add)
        for tt in range(lo, lo + 8):
            tps = ps_kq.tile([P, 2, P], DT, tag="tps", bufs=2,
                             name=f"tpsk{tt}")
            for qc in range(2):
                nc.tensor.transpose(tps[:, qc, :],
                                    kf[:, tt, qc * P:(qc + 1) * P],
                                    ident[:])
            nc.vector.tensor_copy(out=kT[:, :, tt * P:(tt + 1) * P],
                                  in_=tps[:])
            if ep == 0 and tt < NQT:
                tps2 = ps_kq.tile([P, 2, P], DT, tag="tps", bufs=2,
                                  name=f"tpsq{tt}")
                for qc in range(2):
                    nc.tensor.transpose(tps2[:, qc, :],
                                        qf[:, tt, qc * P:(qc + 1) * P],
                                        ident[:])
                nc.vector.tensor_copy(
                    out=qT[:, :, tt * P:(tt + 1) * P], in_=tps2[:])

    with (tc.tile_pool(name="xin", bufs=3) as xin,
          tc.tile_pool(name="emb", bufs=2) as embpool,
          tc.tile_pool(name="ps_a", bufs=2, space="PSUM") as ps_a,
          tc.tile_pool(name="ps_kq", bufs=1, space="PSUM") as ps_kq):
        embp = {"pool": embpool}
        for tb in range(T // 512):
            xt = xin.tile([P, 8, 512], DT, tag="xt", name=f"xt{tb}")
            if tb == 0:
                # split the first block across both HWDGE queues so the
                # leading kv matmuls start ~1.5us sooner
                nc.sync.dma_start(xt[:, 0:4, :],
                                  xTr[:, 0:4, tb * 512:(tb + 1) * 512])
                nc.scalar.dma_start(xt[:, 4:8, :],
                                    xTr[:, 4:8, tb * 512:(tb + 1) * 512])
            else:
                nc.sync.dma_start(xt[:], xTr[:, :, tb * 512:(tb + 1) * 512])
            kvps = ps_a.tile([64, 512], f32, tag="kvps", bufs=1)
            for cc in range(8):
                nc.tensor.matmul(kvps[:], wkv[:, cc, :], xt[:, cc, :],
                                 start=(cc == 0), stop=(cc == 7))
            nc.scalar.copy(out=kvmid[:, tb * 512:(tb + 1) * 512],
                           in_=kvps[:])
            if tb < 2:
                xqt = xin.tile([P, 8, 512], DT, tag="xt", name=f"xqt{tb}")
                nc.scalar.dma_start(xqt[:],
                                    xqTr[:, :, tb * 512:(tb + 1) * 512])
                for j in range(4):
                    tt = tb * 4 + j
                    qps = ps_kq.tile([P, QD], f32, tag="kqps", bufs=2,
                                     name=f"qps{tt}")
                    for cc in range(8):
                        nc.tensor.matmul(qps[:],
                                         xqt[:, cc, j * P:(j + 1) * P],
                                         wqq[:, cc, :],
                                         start=(cc == 0), stop=(cc == 7))
                    nc.scalar.copy(out=qraw[:, tt, :], in_=qps[:])
            if tb == 0:
                xqp = xin.tile([P, 8, NSLOT], DT, tag="xqp")
                nc.sync.dma_start(xqp[:], xqpr[:])
                qpps = ps_kq.tile([NSLOT, QD], f32, tag="kqps", bufs=2,
                                  name="qpps")
                for cc in range(8):
                    nc.tensor.matmul(qpps[:], xqp[:, cc, :], wqq[:, cc, :],
                                     start=(cc == 0), stop=(cc == 7))
                nc.scalar.copy(out=qprev[:], in_=qpps[:])
            for tt in range(4 * tb, 4 * tb + 4):
                k_tile(tt, embp, ps_kq)
                if tt < NQT:
                    q_tile(tt, ps_kq)
            if tb == 1:
                kq_epoch(0, ps_kq)
            if tb == 2:
                vemb_fetch(0)   # prefetch first v-emb group for phase 2
        kq_epoch(1, ps_kq)

    if phases < 2:
        if loop is not None:
            loop.__exit__(None, None, None)
        ctx.close()
        return

    # =============== Phase 2: v chain interleaved with attention ========
    def emit_v(tt, ps_v, vembs):
        j = tt % 4
        vemb1, vemb2 = vembs
        v2_prev = vstate["v2_prev"]
        # vps and shps rotate through one double-buffered full-width tag
        # (4 banks): vps(t+1) waits only on tanh(t), shps(t) on the blend.
        # half-width psum tiles (1 bank each, double-buffered) keep the
        # v chain pipelined across tiles within 4 banks total.
        vt = vwork.tile([P, C], DT, tag="vt", name=f"vt{tt}")
        for h in range(2):
            vps = ps_v.tile([P, 512], f32, tag="vps", bufs=2,
                            name=f"vps{tt}_{h}")
            nc.tensor.matmul(vps[:],
                             kvmid[KV:64, tt * P:(tt + 1) * P],
                             wvup[:, h * 512:(h + 1) * 512],
                             start=True, stop=True)
            nc.scalar.activation(vt[:, h * 512:(h + 1) * 512], vps[:],
                                 Act.Tanh)
        v2 = vwork.tile([P, C], DT, tag="v2", name=f"v2_{tt}")
        nc.gpsimd.tensor_tensor(out=vv[:, tt, :], in0=vt[:],
                                in1=vemb1[:, j, :], op=Alu.mult)
        nc.gpsimd.tensor_tensor(out=v2[:], in0=vt[:],
                                in1=vemb2[:, j, :], op=Alu.mult)
        for h in range(2):
            cs = slice(h * 512, (h + 1) * 512)
            shps = ps_v.tile([P, 512], f32, tag="vsh", bufs=2,
                             name=f"vsh{tt}_{h}")
            nc.tensor.matmul(shps[:], ssup[:], v2[:, cs], start=True,
                             stop=v2_prev is None)
            if v2_prev is not None:
                nc.tensor.matmul(shps[:], bnd[:], v2_prev[:, cs],
                                 start=False, stop=True)
            # blend add + half row-sum in one DVE pass; sumsq in one more
            # (vt is dead here — reuse as the unread elementwise output;
            # sumsq uses scalar_tensor_tensor since tensor_tensor_reduce
            # hangs the DVE on hardware).
            nc.vector.scalar_tensor_tensor(
                out=vv[:, tt, cs], in0=shps[:], scalar=1.0,
                in1=vv[:, tt, cs],
                op0=Alu.mult, op1=Alu.add, accum_out=vst[:, tt, h:h + 1])
            nc.vector.scalar_tensor_tensor(
                out=vt[:, cs], in0=vv[:, tt, cs], scalar=1.0,
                in1=vv[:, tt, cs],
                op0=Alu.mult, op1=Alu.mult,
                accum_out=vst[:, tt, 2 + h:3 + h])
        vstate["v2_prev"] = v2

    def v_group_ln(g):
        # tiles [4g, 4g+4): merge half-sums, batched rsqrt, then LN apply
        # split across Pool and DVE halves (halves the group bubble).
        s4 = slice(4 * g, 4 * g + 4)
        nc.vector.tensor_tensor(out=vst[:, s4, 0:1], in0=vst[:, s4, 0:1],
                                in1=vst[:, s4, 1:2], op=Alu.add)
        nc.vector.tensor_tensor(out=vst[:, s4, 1:2], in0=vst[:, s4, 2:3],
                                in1=vst[:, s4, 3:4], op=Alu.add)
        batch_rsqrt(vst, 4 * g, 4, vwork, f"vrs{g}", inv_w=1.0 / C)
        for tt in range(4 * g, 4 * g + 4):
            nc.vector.tensor_scalar(out=vv[:, tt, :], in0=vv[:, tt, :],
                                    scalar1=vst[:, tt, 0:1],
                                    scalar2=vst[:, tt, 1:2],
                                    op0=Alu.subtract, op1=Alu.mult)
            if gv is not None:
                nc.gpsimd.tensor_tensor(out=vv[:, tt, :], in0=vv[:, tt, :],
                                        in1=gv[:], op=Alu.mult)
                nc.gpsimd.tensor_tensor(out=vv[:, tt, :], in0=vv[:, tt, :],
                                        in1=bv[:], op=Alu.add)

    # L2(s, ch, kp): out-psum accumulation for C-half ch (replays the ee
    # strip) — needs LN'd v tiles < R[s]. 2 psum banks total.
    def attn_l2(s, ch, kp, ps_out, st):
        if ch == 0 and kp == 0:
            st["ot"] = [outsp.tile([P, C], DT, tag=f"ot{i}",
                                   name=f"ot_{s}_{i}") for i in range(2)]
        ee, ot, recip = st[f"ee{s}"], st["ot"], st[f"recip{s}"]
        cs = slice(ch * 512, (ch + 1) * 512)
        last_kp = kp == R[s] // 2 - 1
        for h in range(2):
            kt = 2 * kp + h
            for i in range(2):
                opsi = ps_out.tile([P, 512], f32, tag=f"o{i}", bufs=1,
                                   name=f"ops_{s}_{ch}_{i}") \
                    if kp == 0 and h == 0 else st[f"o{i}"]
                if kp == 0 and h == 0:
                    st[f"o{i}"] = opsi
                nc.tensor.matmul(opsi[:], ee[:, kt, i * P:(i + 1) * P],
                                 vv[:, kt, cs],
                                 start=(kt == 0), stop=(kt == R[s] - 1))
        if last_kp:
            for i in range(2):
                nc.vector.tensor_scalar_mul(
                    out=ot[i][:, cs], in0=st[f"o{i}"][:],
                    scalar1=recip[:, i:i + 1])
            if ch == 1:
                for i in range(2):
                    nc.gpsimd.dma_start(
                        out_d[s * CHUNK + i * P:s * CHUNK + (i + 1) * P, :],
                        ot[i][:])

    with (tc.tile_pool(name="ps_v", bufs=1, space="PSUM") as ps_v,
          tc.tile_pool(name="ps_out", bufs=1, space="PSUM") as ps_out,
          tc.tile_pool(name="ps_sc", bufs=1, space="PSUM") as ps_sc,
          tc.tile_pool(name="ps_sum", bufs=1, space="PSUM") as ps_sum):
        apools["ps_sc"] = ps_sc
        apools["ps_sum"] = ps_sum

        def add_l2(s):
            for ch in range(2):
                for kp in range(R[s] // 2):
                    pending.append(
                        lambda s=s, ch=ch, kp=kp:
                        attn_l2(s, ch, kp, ps_out, astate))

        # all score/exp work only needs kT/qT — queue it up front
        for s in range(NSLOT):
            add_l1(s)
        for tt in range(NT):
            g, j = tt // 4, tt % 4
            if j == 0:
                vembs = (vstate["vemb1"], vstate["vemb2"])
                if g < 3:
                    vemb_fetch(g + 1)   # prefetch next group
            emit_v(tt, ps_v, vembs)
            if j == 3:
                v_group_ln(g)
                add_l2(g)
            emit_items(3 if tt < 4 else 5)
        emit_items(len(pending))

    if loop is not None:
        loop.__exit__(None, None, None)
    ctx.close()


_NC_CACHE = {}


def _input_specs(apply_gb, bf16):
    import concourse.mybir as mybir
    f32 = mybir.dt.float32
    DT = mybir.dt.bfloat16 if bf16 else f32
    specs = [
        ("xT", [C, T], DT), ("xqT", [C, TQ], DT),
        ("xqprevT", [C, NSLOT], DT),
        ("kemb", [T, QD], DT), ("vemb1", [T, C], DT),
        ("vemb2", [T, C], DT),
        ("wqq", [C, QD], DT), ("wkv", [C, 64], DT),
        ("wkup", [KV, QD], DT), ("wvup", [KV, C], DT),
        ("xq_rep", [P, QD], DT), ("xk_rep", [P, QD], DT),
        ("mask", [NMASK, P, CHUNK], DT),
    ]
    if apply_gb:
        specs += [("gq_rep", [P, QD], DT), ("bq_rep", [P, QD], DT),
                  ("gk_rep", [P, QD], DT), ("bk_rep", [P, QD], DT),
                  ("gv_rep", [P, C], DT), ("bv_rep", [P, C], DT)]
    return specs


def get_nc(apply_gb, bf16=True, nrep=1, phases=4):
    key = (bool(apply_gb), bool(bf16), int(nrep), int(phases))
    if key in _NC_CACHE:
        return _NC_CACHE[key]
    import concourse.mybir as mybir
    import concourse.tile as tile
    from concourse import bacc

    nc = bacc.Bacc("TRN2", target_bir_lowering=False, debug=False,
                   num_devices=N_CORES)
    a = {}
    for name, shape, dt in _input_specs(apply_gb, bf16):
        a[name] = nc.dram_tensor(name, shape, dt, kind="ExternalInput").ap()
    DT = mybir.dt.bfloat16 if bf16 else mybir.dt.float32
    a["out"] = nc.dram_tensor("out", [TQ, C], DT,
                              kind="ExternalOutput").ap()
    with tile.TileContext(nc) as tc:
        _build_program(nc, tc, a, apply_gb, bf16, nrep=nrep, phases=phases)
    nc.compile()
    _NC_CACHE[key] = nc
    return nc


def _parity_mask(parity):
    m = np.zeros((NMASK, P, CHUNK), np.float32)
    for (s, kt), mi in MASK_IDX.items():
        qs = CHUNKS[parity][s] * CHUNK
        kg = np.arange(P, dtype=np.int64)[:, None] + P * kt
        qg = np.arange(CHUNK, dtype=np.int64)[None, :] + qs
        m[mi] = np.where(qg >= kg, 0.0, NEG).astype(np.float32)
    return m


def make_in_maps(inputs, bf16=True):
    import ml_dtypes
    cdt = ml_dtypes.bfloat16 if bf16 else np.float32

    x = np.asarray(inputs["x"], np.float32)
    idx = np.asarray(inputs["idx"]).astype(np.int64)
    k_tab = np.asarray(inputs["k_emb_tab"], np.float32)
    v_tab = np.asarray(inputs["v_emb_tab"], np.float32)
    W_qq = np.asarray(inputs["W_qq"], np.float32)
    W_k = np.asarray(inputs["W_k"], np.float32)
    W_kup = np.asarray(inputs["W_kup"], np.float32)
    W_v = np.asarray(inputs["W_v"], np.float32)
    W_vup = np.asarray(inputs["W_vup"], np.float32)
    x_q = np.asarray(inputs["x_q"], np.float32).reshape(QD)
    x_k = np.asarray(inputs["x_k"], np.float32).reshape(QD)
    x_v = np.asarray(inputs["x_v"], np.float32).reshape(C)
    g_q = np.asarray(inputs["g_q"], np.float32).reshape(QD)
    b_q = np.asarray(inputs["b_q"], np.float32).reshape(QD)
    g_k = np.asarray(inputs["g_k"], np.float32).reshape(QD)
    b_k = np.asarray(inputs["b_k"], np.float32).reshape(QD)
    g_v = np.asarray(inputs["g_v"], np.float32).reshape(C)
    b_v = np.asarray(inputs["b_v"], np.float32).reshape(C)

    apply_gb = not (np.all(g_q == 1) and np.all(b_q == 0)
                    and np.all(g_k == 1) and np.all(b_k == 0)
                    and np.all(g_v == 1) and np.all(b_v == 0))

    k_emb = k_tab[idx]          # [B, T, QD]
    v_emb = v_tab[idx]          # [B, T, C]
    vemb1 = [np.ascontiguousarray(v_emb[b] * (1.0 - x_v)).astype(cdt)
             for b in range(B)]
    vemb2 = [np.ascontiguousarray(v_emb[b] * x_v).astype(cdt)
             for b in range(B)]

    def cvt(arr):
        return np.ascontiguousarray(arr).astype(cdt)

    shared = {
        "wqq": cvt(W_qq.T),
        "wkv": cvt(np.concatenate([W_k, W_v], 0).T),
        "wkup": cvt(W_kup.T),
        "wvup": cvt(W_vup.T),
        "xq_rep": cvt(np.broadcast_to(x_q, (P, QD))),
        "xk_rep": cvt(np.broadcast_to(x_k, (P, QD))),
    }
    if apply_gb:
        for nm, v in [("gq", g_q), ("bq", b_q), ("gk", g_k), ("bk", b_k)]:
            shared[nm + "_rep"] = cvt(np.broadcast_to(v, (P, QD)))
        for nm, v in [("gv", g_v), ("bv", b_v)]:
            shared[nm + "_rep"] = cvt(np.broadcast_to(v, (P, C)))

    pmask = [_parity_mask(0).astype(cdt), _parity_mask(1).astype(cdt)]
    in_maps = []
    for c in range(N_CORES):
        b, parity = c // 2, c % 2
        chunks = CHUNKS[parity]
        cols = np.concatenate([np.arange(ch * CHUNK, (ch + 1) * CHUNK)
                               for ch in chunks])
        xqprev = np.zeros((NSLOT, C), np.float32)
        for j, ch in enumerate(chunks):
            if ch > 0:
                xqprev[j] = x[b, ch * CHUNK - 1]
        m = dict(shared)
        m.update(
            xT=cvt(x[b].T), xqT=cvt(x[b][cols].T),
            xqprevT=cvt(xqprev.T),
            kemb=cvt(k_emb[b]),
            vemb1=vemb1[b], vemb2=vemb2[b],
            mask=pmask[parity],
        )
        in_maps.append(m)
    return in_maps, apply_gb


def assemble_output(results):
    out = np.empty((B, T, C), np.float32)
    for c in range(N_CORES):
        oc = np.asarray(results[c]["out"]).astype(np.float32)
        for j, ch in enumerate(CHUNKS[c % 2]):
            out[c // 2, ch * CHUNK:(ch + 1) * CHUNK] = \
                oc[j * CHUNK:(j + 1) * CHUNK]
    return out


BF16 = True


def kernel(**inputs):
    from concourse.bass_utils import run_bass_kernel_spmd
    in_maps, apply_gb = make_in_maps(inputs, bf16=BF16)
    nc = get_nc(apply_gb, bf16=BF16)
    res = run_bass_kernel_spmd(nc, in_maps, core_ids=list(range(N_CORES)))
    return assemble_output(res.results)
